# revision 51
# baseline (speedup 1.0000x reference)
"""Trainium2 Bass kernel for nn_AttentionBlock (ragged_sequence, 16 equal
segments of 2048 q/kv tokens, HID=256, QD=64) on 8 NeuronCores.

Sharding: 2 segments (4096 rows) per core, weights replicated, outputs
concatenated host-side (attention is block-diagonal per segment -> no
cross-core communication needed).

All attention math (q/k/v projections, scores, probs@V) runs in fp8e4m3
with DoubleRow matmuls (2 k-tiles per pass); FC + layernorms stay
bf16/f32.  Host pre-scales WQ/WK/WV by 8 so fp8 operands sit mid-range;
the score scale and the V "ones column" (=8) cancel it exactly.
"""

import os
import sys

os.environ.setdefault("MYCRO_LOCAL_CACHE", "1")
if "/opt/trn_rl_repo" not in sys.path:
    sys.path.insert(0, "/opt/trn_rl_repo")

import numpy as np

HID = 256
QD = 64
LQ = 2048
LH = 2048
B = 16
NCORES = 8
SEGS = 2                  # segments per core
ROWS = SEGS * LQ          # 4096 q rows per core
EPS = 1e-5
WSC = 8.0                 # host-side WQ/WK/WV pre-scale for fp8 range
SCALE = 1.0 / (8.0 * WSC * WSC)   # 1/sqrt(QD), WQ/WK scales cancelled

_built = {}               # (apply0,) -> nc


def _patch_act_tables():
    """Make the act-table pass choose the combined exp+ln table for every
    activation: blank all other tables (indices preserved so walrus's
    act_func_set_id remap stays correct). Avoids 100+ ACT_TABLE_LOADs
    (1.28us each) from alternating Exp/Ln table picks."""
    import functools
    import concourse.hw_specs as hw_specs
    import concourse.bacc as bacc_mod
    if getattr(hw_specs, "_attn_tables_patched", False):
        return
    orig = hw_specs.get_activation_tables

    @functools.cache
    def patched(arch):
        tabs = dict(orig(arch))
        joint = "natural_log_exp_and_others"
        assert joint in tabs, sorted(tabs)
        return {name: (funcs if name == joint else set())
                for name, funcs in tabs.items()}

    hw_specs.get_activation_tables = patched
    bacc_mod.get_activation_tables = patched
    hw_specs._attn_tables_patched = True


def _build(apply0: bool):
    from concourse import bacc, bass, mybir, tile

    _patch_act_tables()

    dt = mybir.dt
    f32 = dt.float32
    bf16 = dt.bfloat16
    f8 = dt.float8e4
    AF = mybir.ActivationFunctionType
    Alu = mybir.AluOpType
    DR = mybir.MatmulPerfMode.DoubleRow

    NJT = LH // 128           # 16 j-tiles per segment
    NJP = NJT // 2            # 8 j-tile pairs
    NIC = 2                   # 1024-col i-chunks per segment
    ICW = LQ // NIC           # 1024
    NIL = ICW // 128          # 8 i-tiles per chunk
    GRP = 4                   # layernorm stats group (i-tiles)
    VB = HID + 1              # V block width incl ones column
    SEGROWS = ROWS // SEGS    # 2048

    nc = bacc.Bacc("TRN2", target_bir_lowering=False, debug=False,
                   enable_asserts=False)

    qT_d = nc.dram_tensor("qT", [HID, ROWS], f8, kind="ExternalInput")
    q_d = nc.dram_tensor("q", [ROWS, HID], bf16, kind="ExternalInput")
    hT_d = nc.dram_tensor("hT", [HID, ROWS], f8, kind="ExternalInput")
    wqT_d = nc.dram_tensor("WQT", [HID, QD], f8, kind="ExternalInput")
    wkT_d = nc.dram_tensor("WKT", [HID, QD], f8, kind="ExternalInput")
    wvT_d = nc.dram_tensor("WVT", [HID, HID], f8, kind="ExternalInput")
    fwT_d = nc.dram_tensor("FCWT", [HID, HID], bf16, kind="ExternalInput")
    fb_d = nc.dram_tensor("FCB", [1, HID], bf16, kind="ExternalInput")
    if apply0:
        n0w_d = nc.dram_tensor("N0W", [128, HID], f32, kind="ExternalInput")
        n0b_d = nc.dram_tensor("N0B", [128, HID], f32, kind="ExternalInput")
    out_d = nc.dram_tensor("out", [ROWS, HID], f32, kind="ExternalOutput")

    qT_a, q_a, hT_a = qT_d.ap(), q_d.ap(), hT_d.ap()
    out_a = out_d.ap()

    def pair2(ap2d):
        """[p, 2*W] -> [p, 2, W] view for DoubleRow operands."""
        return ap2d.rearrange("p (two w) -> p two w", two=2)

    with tile.TileContext(nc) as tc:
        with (
            tc.tile_pool(name="const", bufs=1) as cpool,
            tc.tile_pool(name="kqq", bufs=1) as kqq_pool,
            tc.tile_pool(name="vsb", bufs=1) as v_pool,
            tc.tile_pool(name="qhT", bufs=2) as qh_pool,
            tc.tile_pool(name="pt", bufs=3) as pt_pool,
            tc.tile_pool(name="qrow", bufs=4) as q_pool,
            tc.tile_pool(name="ep", bufs=6) as ep_pool,
            tc.tile_pool(name="ep8", bufs=10) as ep8_pool,
            tc.tile_pool(name="st8", bufs=12) as st8_pool,
            tc.tile_pool(name="outp", bufs=4) as o_pool,
            tc.tile_pool(name="ps_st", bufs=2,
                         space=bass.MemorySpace.PSUM) as ps_st,
            tc.tile_pool(name="ps_att", bufs=3,
                         space=bass.MemorySpace.PSUM) as ps_att,
            tc.tile_pool(name="ps_fc", bufs=1,
                         space=bass.MemorySpace.PSUM) as ps_fc,
        ):
            # ---- constants ----
            wq_sb = cpool.tile([128, 2 * QD], f8)      # [e, (chunk, c)]
            wk_sb = cpool.tile([128, 2 * QD], f8)
            wv_sb = cpool.tile([128, 2 * HID], f8)     # [e, (chunk, d)]
            fw_sb = cpool.tile([128, 2 * HID], bf16)   # fc_w.T chunks
            fb_sb = cpool.tile([1, HID], bf16)
            one_sb = cpool.tile([1, 128], bf16)
            nc.vector.memset(one_sb[:], 1.0)
            for e in range(2):
                nc.sync.dma_start(wq_sb[:, e * QD:(e + 1) * QD],
                                  wqT_d.ap()[e * 128:(e + 1) * 128, :])
                nc.sync.dma_start(wk_sb[:, e * QD:(e + 1) * QD],
                                  wkT_d.ap()[e * 128:(e + 1) * 128, :])
                nc.sync.dma_start(wv_sb[:, e * HID:(e + 1) * HID],
                                  wvT_d.ap()[e * 128:(e + 1) * 128, :])
                nc.sync.dma_start(fw_sb[:, e * HID:(e + 1) * HID],
                                  fwT_d.ap()[e * 128:(e + 1) * 128, :])
            nc.sync.dma_start(fb_sb[:], fb_d.ap()[:, :])
            eps_sb = cpool.tile([128, 1], f32)
            nc.vector.memset(eps_sb[:], EPS)
            # exp bias: p = exp(score*scale - 1) centers p in fp8e4m3 range
            nb1_sb = cpool.tile([128, 1], f32)
            nc.vector.memset(nb1_sb[:], -1.0)
            if apply0:
                n0w_sb = cpool.tile([128, HID], f32)
                n0b_sb = cpool.tile([128, HID], f32)
                nc.sync.dma_start(n0w_sb[:], n0w_d.ap()[:, :])
                nc.sync.dma_start(n0b_sb[:], n0b_d.ap()[:, :])

            # persistent activations
            kT_sb = kqq_pool.tile([64, ROWS], f8)       # K^T  [c, j]
            qq_sb = kqq_pool.tile([64, ROWS], f8)       # qq^T [c, i]
            v_sb = v_pool.tile([128, SEGS * NJT * VB], f8)
            # every V block's ones column, set once
            nc.gpsimd.memset(
                v_sb[:, :].rearrange("p (b w) -> p b w", w=VB)
                [:, :, HID:HID + 1], WSC)

            def load_segment(s):
                """DMA qT/hT (fp8) tiles for segment s."""
                srow = s * SEGROWS
                qts, hts = [], []
                for k in range(2):          # 1024-col spans
                    span = slice(srow + k * 1024, srow + (k + 1) * 1024)
                    tq = qh_pool.tile([128, 2048], f8, tag=f"qt{k}")
                    th = qh_pool.tile([128, 2048], f8, tag=f"ht{k}")
                    for e in range(2):
                        nc.sync.dma_start(
                            tq[:, e * 1024:(e + 1) * 1024],
                            qT_a[e * 128:(e + 1) * 128, span])
                        nc.sync.dma_start(
                            th[:, e * 1024:(e + 1) * 1024],
                            hT_a[e * 128:(e + 1) * 128, span])
                    qts.append(tq)
                    hts.append(th)
                return qts, hts

            cp_eng = [nc.vector.tensor_copy, nc.scalar.copy]

            def proj_kqq_span(s, dst, w_sb, tiles, sp):
                """one 512-col kT/qq span: single DoubleRow matmul."""
                srow = s * SEGROWS
                k, off = divmod(sp * 512, 1024)
                ps = ps_st.tile([64, 512], f32, tag="st", name="kqp")
                nc.tensor.matmul(
                    ps[:], pair2(w_sb[:, :]),
                    pair2(tiles[k][:, :])[:, :, off:off + 512],
                    start=True, stop=True, perf_mode=DR)
                col = srow + sp * 512
                cp_eng[sp % 2](dst[:, col:col + 512], ps[:])

            def project_part1(s, qts, hts):
                """kT (all spans) + qq spans 0-1: enough for ic=0 scores."""
                for sp in range(4):
                    proj_kqq_span(s, kT_sb, wk_sb, hts, sp)
                for sp in range(2):
                    proj_kqq_span(s, qq_sb, wq_sb, qts, sp)

            def project_part2(s, qts, hts):
                """qq spans 2-3 + V projection."""
                for sp in range(2, 4):
                    proj_kqq_span(s, qq_sb, wq_sb, qts, sp)
                vcp = [nc.vector.tensor_copy, nc.scalar.copy]
                for jt in range(NJT):
                    ps = ps_att.tile([128, HID], f32, tag="att", name="vp")
                    k, off = divmod(jt * 128, 1024)
                    nc.tensor.matmul(
                        ps[:],
                        pair2(hts[k][:, :])[:, :, off:off + 128],
                        pair2(wv_sb[:, :]),
                        start=True, stop=True, perf_mode=DR)
                    base = (s * NJT + jt) * VB
                    vcp[jt % 2](v_sb[:, base:base + HID], ps[:])

            def scores_chunk(s, ic):
                """scores^T (bf16) + exp -> P^T fp8 pair tile."""
                srow = s * SEGROWS
                icol = srow + ic * ICW
                pt_all = pt_pool.tile([128, NJT * ICW], f8, tag="pt")
                for jt in range(NJT):
                    st = ps_st.tile([128, ICW], f32, tag="st")
                    jcol = s * LH + jt * 128
                    for h in range(2):
                        nc.tensor.matmul(
                            st[:, h * 512:(h + 1) * 512],
                            kT_sb[:, jcol:jcol + 128],
                            qq_sb[:, icol + h * 512:icol + (h + 1) * 512],
                            start=True, stop=True)
                    nc.scalar.activation(
                        pt_all[:, jt * ICW:(jt + 1) * ICW], st[:],
                        AF.Exp, scale=SCALE, bias=nb1_sb[:])
                return pt_all

            def attv_pair(s, pt_all, il_a):
                """two interleaved att@V chains (hides ldweights)."""
                atts = [ps_att.tile([128, VB], f32, tag="att",
                                    name=f"att{u}") for u in range(2)]
                for t in range(NJP):
                    vb = (s * NJT + 2 * t) * VB
                    vv = pair2(v_sb[:, vb:vb + 2 * VB])
                    for u in range(2):
                        il = il_a + u
                        nc.tensor.matmul(
                            atts[u][:],
                            pair2(pt_all[:, 2 * t * ICW:(2 * t + 2) * ICW])
                            [:, :, il * 128:(il + 1) * 128],
                            vv,
                            start=(t == 0), stop=(t == NJP - 1),
                            perf_mode=DR)
                return atts

            def x_residual(att, row0, mva0, gi):
                """x0 = att/den + q, plus running LN stats."""
                qt = q_pool.tile([128, HID], bf16, tag="q")
                nc.sync.dma_start(qt[:], q_a[row0:row0 + 128, :])
                rden = ep8_pool.tile([128, 1], f32, tag="rd")
                nc.vector.reciprocal(rden[:], att[:, HID:HID + 1])
                x0 = ep8_pool.tile([128, HID], f32, tag="x0")
                nc.vector.scalar_tensor_tensor(
                    x0[:], att[:, 0:HID], rden[:].opt(), qt[:],
                    op0=Alu.mult, op1=Alu.add)
                mv6 = st8_pool.tile([128, 6], f32, tag="mv6")
                nc.vector.bn_stats(mv6[:], x0[:])
                nc.vector.bn_aggr(mva0[:, 2 * gi:2 * gi + 2], mv6[:])
                return x0

            def ln_rstd(mva, glen, tag):
                ln8 = st8_pool.tile([128, glen], f32, tag=f"ln{tag}")
                nc.scalar.activation(
                    ln8[:].rearrange("p (t o) -> p t o", o=1),
                    mva[:].rearrange("p (t o) -> p t o", o=2)[:, :, 1:2],
                    AF.Ln, bias=eps_sb[:])
                rstd8 = st8_pool.tile([128, glen], f32, tag=f"r{tag}")
                nc.scalar.activation(rstd8[:], ln8[:], AF.Exp, scale=-0.5)
                return rstd8

            def fc_block(x0, mva0, rstd8a, mva1, gi):
                """z = LN0(x0); hres = z@fcw+fb; y0 = relu(hres)+z."""
                z = ep8_pool.tile([128, HID], bf16, tag="z")
                nc.vector.tensor_scalar(
                    z[:], x0[:], mva0[:, 2 * gi:2 * gi + 1].opt(),
                    rstd8a[:, gi:gi + 1].opt(),
                    op0=Alu.subtract, op1=Alu.mult)
                if apply0:
                    z2 = ep_pool.tile([128, HID], bf16, tag="z2")
                    nc.gpsimd.tensor_tensor(z2[:], z[:], n0w_sb[:],
                                            op=Alu.mult)
                    z3 = ep_pool.tile([128, HID], bf16, tag="z3")
                    nc.gpsimd.tensor_tensor(z3[:], z2[:], n0b_sb[:],
                                            op=Alu.add)
                    zf = z3
                else:
                    zf = z
                hres = ps_fc.tile([128, HID], f32, tag="fc")
                nc.tensor.matmul(hres[:], one_sb[:], fb_sb[:],
                                 start=True, stop=False)
                for hh in range(2):
                    zT = ep_pool.tile([128, 128], bf16, tag=f"zT{hh}")
                    nc.sync.dma_start_transpose(
                        zT[:], zf[:, hh * 128:(hh + 1) * 128])
                    nc.tensor.matmul(
                        hres[:], zT[:], fw_sb[:, hh * HID:(hh + 1) * HID],
                        start=False, stop=(hh == 1))
                y0 = ep8_pool.tile([128, HID], f32, tag="y0")
                nc.vector.scalar_tensor_tensor(
                    y0[:], hres[:], 0.0, zf[:], op0=Alu.max, op1=Alu.add)
                mv6b = st8_pool.tile([128, 6], f32, tag="mv6b")
                nc.vector.bn_stats(mv6b[:], y0[:])
                nc.vector.bn_aggr(mva1[:, 2 * gi:2 * gi + 2], mv6b[:])
                return y0

            def store_out(y0, mva1, rstd8b, gi, row0):
                b1 = st8_pool.tile([128, 1], f32, tag="b1")
                nc.vector.tensor_scalar(
                    b1[:], mva1[:, 2 * gi:2 * gi + 1],
                    rstd8b[:, gi:gi + 1].opt(), -1.0,
                    op0=Alu.mult, op1=Alu.mult)
                ot = o_pool.tile([128, HID], f32, tag="ot")
                nc.scalar.activation(
                    ot[:], y0[:], AF.Identity,
                    bias=b1[:], scale=rstd8b[:, gi:gi + 1].opt())
                nc.sync.dma_start(out_a[row0:row0 + 128, :], ot[:])

            def attn_group(s, ic, g0, glen, pt_all):
                srow = s * SEGROWS
                mva0 = st8_pool.tile([128, 2 * glen], f32, tag="mva0")
                xs = []
                for il2 in range(glen // 2):
                    il_a = g0 + 2 * il2
                    atts = attv_pair(s, pt_all, il_a)
                    for u in range(2):
                        il = il_a + u
                        row0 = srow + (ic * NIL + il) * 128
                        xs.append(x_residual(atts[u], row0, mva0,
                                             2 * il2 + u))
                rstd8a = ln_rstd(mva0, glen, "8a")
                mva1 = st8_pool.tile([128, 2 * glen], f32, tag="mva1")
                ys = [fc_block(xs[gi], mva0, rstd8a, mva1, gi)
                      for gi in range(glen)]
                rstd8b = ln_rstd(mva1, glen, "8b")
                for gi in range(glen):
                    row0 = srow + (ic * NIL + g0 + gi) * 128
                    store_out(ys[gi], mva1, rstd8b, gi, row0)

            for s in range(SEGS):
                qts, hts = load_segment(s)
                project_part1(s, qts, hts)
                pt0 = scores_chunk(s, 0)
                project_part2(s, qts, hts)
                last = s == SEGS - 1
                for ic in range(NIC):
                    pt_all = pt0 if ic == 0 else scores_chunk(s, ic)
                    if last and ic == NIC - 1:
                        groups = [(0, 4), (4, 2), (6, 2)]
                    else:
                        groups = [(g * GRP, GRP) for g in range(NIL // GRP)]
                    for g0, glen in groups:
                        attn_group(s, ic, g0, glen, pt_all)

    nc.compile()
    return nc


def _get_nc(apply0: bool):
    key = (bool(apply0),)
    if key not in _built:
        _built[key] = _build(apply0)
    return _built[key]


def _shard(inputs, apply0):
    from concourse import mybir
    bf = mybir.dt.np(mybir.dt.bfloat16)
    f8np = mybir.dt.np(mybir.dt.float8e4)

    q = np.ascontiguousarray(np.asarray(inputs["q"], dtype=np.float32))
    h = np.ascontiguousarray(np.asarray(inputs["h"], dtype=np.float32))
    WQ = np.asarray(inputs["WQ"], dtype=np.float32) * WSC
    WK = np.asarray(inputs["WK"], dtype=np.float32) * WSC
    WV = np.asarray(inputs["WV"], dtype=np.float32) * WSC
    fcw = np.asarray(inputs["fc_w"], dtype=np.float32)
    fcb = np.asarray(inputs["fc_b"], dtype=np.float32)

    WQT = np.ascontiguousarray(WQ.T).astype(f8np)
    WKT = np.ascontiguousarray(WK.T).astype(f8np)
    WVT = np.ascontiguousarray(WV.T).astype(f8np)
    FCWT = np.ascontiguousarray(fcw.T).astype(bf)
    FCB = np.ascontiguousarray(fcb.reshape(1, HID)).astype(bf)

    in_maps = []
    for c in range(NCORES):
        sl = slice(c * ROWS, (c + 1) * ROWS)
        qc = q[sl]
        m = {
            "qT": np.ascontiguousarray(qc.T).astype(f8np),
            "q": qc.astype(bf),
            "hT": np.ascontiguousarray(h[sl].T).astype(f8np),
            "WQT": WQT, "WKT": WKT, "WVT": WVT,
            "FCWT": FCWT, "FCB": FCB,
        }
        if apply0:
            m["N0W"] = np.ascontiguousarray(
                np.broadcast_to(np.asarray(inputs["norm0_w"], np.float32),
                                (128, HID)))
            m["N0B"] = np.ascontiguousarray(
                np.broadcast_to(np.asarray(inputs["norm0_b"], np.float32),
                                (128, HID)))
        in_maps.append(m)
    return in_maps


def _run(inputs, trace=False, tmpdir=None):
    from concourse import bass_utils

    n0w = np.asarray(inputs["norm0_w"], np.float32)
    n0b = np.asarray(inputs["norm0_b"], np.float32)
    n1w = np.asarray(inputs["norm1_w"], np.float32)
    n1b = np.asarray(inputs["norm1_b"], np.float32)
    apply0 = not (np.allclose(n0w, 1.0) and np.allclose(n0b, 0.0))
    apply1 = not (np.allclose(n1w, 1.0) and np.allclose(n1b, 0.0))

    nc = _get_nc(apply0)
    in_maps = _shard(inputs, apply0)
    res = bass_utils.run_bass_kernel_spmd(
        nc, in_maps, core_ids=list(range(NCORES)), trace=trace,
        tmpdir=tmpdir)
    out = np.concatenate([np.asarray(res.results[c]["out"])
                          for c in range(NCORES)], axis=0)
    if apply1:
        out = out * n1w[None, :] + n1b[None, :]
    return out.astype(np.float32), res


def kernel(**inputs):
    out, _ = _run(inputs, trace=False)
    return out


# revision 57
# speedup vs baseline: 1.1485x; 1.1485x over previous
"""Trainium2 Bass kernel for nn_AttentionBlock (ragged_sequence, 16 equal
segments of 2048 q/kv tokens, HID=256, QD=64) on 8 NeuronCores.

Sharding: 2 segments (4096 rows) per core, weights replicated, outputs
concatenated host-side (attention is block-diagonal per segment -> no
cross-core communication needed).

All attention math (q/k/v projections, scores, probs@V) runs in fp8e4m3
with DoubleRow matmuls (2 k-tiles per pass); FC + layernorms stay
bf16/f32.  Host pre-scales WQ/WK/WV by 8 so fp8 operands sit mid-range;
the score scale and the V "ones column" (=8) cancel it exactly.
"""

import os
import sys

os.environ.setdefault("MYCRO_LOCAL_CACHE", "1")
if "/opt/trn_rl_repo" not in sys.path:
    sys.path.insert(0, "/opt/trn_rl_repo")

import numpy as np

HID = 256
QD = 64
LQ = 2048
LH = 2048
B = 16
NCORES = 8
SEGS = 2                  # segments per core
ROWS = SEGS * LQ          # 4096 q rows per core
EPS = 1e-5
WSC = 8.0                 # host-side WQ/WK/WV pre-scale for fp8 range
SCALE = 1.0 / (8.0 * WSC * WSC)   # 1/sqrt(QD), WQ/WK scales cancelled

_built = {}               # (apply0,) -> nc


def _patch_act_tables():
    """Make the act-table pass choose the combined exp+ln table for every
    activation: blank all other tables (indices preserved so walrus's
    act_func_set_id remap stays correct). Avoids 100+ ACT_TABLE_LOADs
    (1.28us each) from alternating Exp/Ln table picks."""
    import functools
    import concourse.hw_specs as hw_specs
    import concourse.bacc as bacc_mod
    if getattr(hw_specs, "_attn_tables_patched", False):
        return
    orig = hw_specs.get_activation_tables

    @functools.cache
    def patched(arch):
        tabs = dict(orig(arch))
        joint = "natural_log_exp_and_others"
        assert joint in tabs, sorted(tabs)
        return {name: (funcs if name == joint else set())
                for name, funcs in tabs.items()}

    hw_specs.get_activation_tables = patched
    bacc_mod.get_activation_tables = patched
    hw_specs._attn_tables_patched = True


def _build(apply0: bool):
    from concourse import bacc, bass, mybir, tile

    _patch_act_tables()

    dt = mybir.dt
    f32 = dt.float32
    bf16 = dt.bfloat16
    f8 = dt.float8e4
    AF = mybir.ActivationFunctionType
    Alu = mybir.AluOpType
    DR = mybir.MatmulPerfMode.DoubleRow

    NJT = LH // 128           # 16 j-tiles per segment
    NJP = NJT // 2            # 8 j-tile pairs
    NIC = 2                   # 1024-col i-chunks per segment
    ICW = LQ // NIC           # 1024
    NIL = ICW // 128          # 8 i-tiles per chunk
    GRP = 4                   # layernorm stats group (i-tiles)
    VB = HID + 1              # V block width incl ones column
    SEGROWS = ROWS // SEGS    # 2048

    nc = bacc.Bacc("TRN2", target_bir_lowering=False, debug=False,
                   enable_asserts=False)

    qT_d = nc.dram_tensor("qT", [HID, ROWS], f8, kind="ExternalInput")
    q_d = nc.dram_tensor("q", [ROWS, HID], bf16, kind="ExternalInput")
    hT_d = nc.dram_tensor("hT", [HID, ROWS], f8, kind="ExternalInput")
    wqT_d = nc.dram_tensor("WQT", [HID, QD], f8, kind="ExternalInput")
    wkT_d = nc.dram_tensor("WKT", [HID, QD], f8, kind="ExternalInput")
    wvT_d = nc.dram_tensor("WVT", [HID, HID], f8, kind="ExternalInput")
    fwT_d = nc.dram_tensor("FCWT", [HID, HID], bf16, kind="ExternalInput")
    fb_d = nc.dram_tensor("FCB", [1, HID], bf16, kind="ExternalInput")
    idt_d = nc.dram_tensor("IDT", [128, 128], bf16, kind="ExternalInput")
    if apply0:
        n0w_d = nc.dram_tensor("N0W", [128, HID], f32, kind="ExternalInput")
        n0b_d = nc.dram_tensor("N0B", [128, HID], f32, kind="ExternalInput")
    out_d = nc.dram_tensor("out", [ROWS, HID], f32, kind="ExternalOutput")

    qT_a, q_a, hT_a = qT_d.ap(), q_d.ap(), hT_d.ap()
    out_a = out_d.ap()

    def pair2(ap2d):
        """[p, 2*W] -> [p, 2, W] view for DoubleRow operands."""
        return ap2d.rearrange("p (two w) -> p two w", two=2)

    with tile.TileContext(nc) as tc:
        with (
            tc.tile_pool(name="const", bufs=1) as cpool,
            tc.tile_pool(name="kqq", bufs=1) as kqq_pool,
            tc.tile_pool(name="vsb", bufs=1) as v_pool,
            tc.tile_pool(name="qhT", bufs=2) as qh_pool,
            tc.tile_pool(name="pt", bufs=3) as pt_pool,
            tc.tile_pool(name="qrow", bufs=4) as q_pool,
            tc.tile_pool(name="ep", bufs=6) as ep_pool,
            tc.tile_pool(name="ep8", bufs=10) as ep8_pool,
            tc.tile_pool(name="st8", bufs=12) as st8_pool,
            tc.tile_pool(name="outp", bufs=4) as o_pool,
            tc.tile_pool(name="ps_st", bufs=2,
                         space=bass.MemorySpace.PSUM) as ps_st,
            tc.tile_pool(name="ps_att", bufs=2,
                         space=bass.MemorySpace.PSUM) as ps_att,
            tc.tile_pool(name="ps_fc", bufs=1,
                         space=bass.MemorySpace.PSUM) as ps_fc,
            tc.tile_pool(name="ps_tp", bufs=1,
                         space=bass.MemorySpace.PSUM) as ps_tp,
        ):
            # ---- constants ----
            wq_sb = cpool.tile([128, 2 * QD], f8)      # [e, (chunk, c)]
            wk_sb = cpool.tile([128, 2 * QD], f8)
            wv_sb = cpool.tile([128, 2 * HID], f8)     # [e, (chunk, d)]
            fw_sb = cpool.tile([128, 2 * HID], bf16)   # fc_w.T chunks
            fb_sb = cpool.tile([1, HID], bf16)
            one_sb = cpool.tile([1, 128], bf16)
            nc.vector.memset(one_sb[:], 1.0)
            idt_sb = cpool.tile([128, 128], bf16)
            nc.sync.dma_start(idt_sb[:], idt_d.ap()[:, :])
            for e in range(2):
                nc.sync.dma_start(wq_sb[:, e * QD:(e + 1) * QD],
                                  wqT_d.ap()[e * 128:(e + 1) * 128, :])
                nc.sync.dma_start(wk_sb[:, e * QD:(e + 1) * QD],
                                  wkT_d.ap()[e * 128:(e + 1) * 128, :])
                nc.sync.dma_start(wv_sb[:, e * HID:(e + 1) * HID],
                                  wvT_d.ap()[e * 128:(e + 1) * 128, :])
                nc.sync.dma_start(fw_sb[:, e * HID:(e + 1) * HID],
                                  fwT_d.ap()[e * 128:(e + 1) * 128, :])
            nc.sync.dma_start(fb_sb[:], fb_d.ap()[:, :])
            eps_sb = cpool.tile([128, 1], f32)
            nc.vector.memset(eps_sb[:], EPS)
            # exp bias: p = exp(score*scale - 1) centers p in fp8e4m3 range
            nb1_sb = cpool.tile([128, 1], f32)
            nc.vector.memset(nb1_sb[:], -1.0)
            if apply0:
                n0w_sb = cpool.tile([128, HID], f32)
                n0b_sb = cpool.tile([128, HID], f32)
                nc.sync.dma_start(n0w_sb[:], n0w_d.ap()[:, :])
                nc.sync.dma_start(n0b_sb[:], n0b_d.ap()[:, :])

            # persistent activations
            kT_sb = kqq_pool.tile([64, ROWS], f8)       # K^T  [c, j]
            qq_sb = kqq_pool.tile([64, ROWS], f8)       # qq^T [c, i]
            v_sb = v_pool.tile([128, SEGS * NJT * VB], f8)
            # every V block's ones column, set once
            nc.gpsimd.memset(
                v_sb[:, :].rearrange("p (b w) -> p b w", w=VB)
                [:, :, HID:HID + 1], WSC)

            def load_segment(s):
                """DMA qT/hT (fp8) tiles for segment s."""
                srow = s * SEGROWS
                qts, hts = [], []
                for k in range(2):          # 1024-col spans
                    span = slice(srow + k * 1024, srow + (k + 1) * 1024)
                    tq = qh_pool.tile([128, 2048], f8, tag=f"qt{k}")
                    th = qh_pool.tile([128, 2048], f8, tag=f"ht{k}")
                    for e in range(2):
                        nc.sync.dma_start(
                            tq[:, e * 1024:(e + 1) * 1024],
                            qT_a[e * 128:(e + 1) * 128, span])
                        nc.sync.dma_start(
                            th[:, e * 1024:(e + 1) * 1024],
                            hT_a[e * 128:(e + 1) * 128, span])
                    qts.append(tq)
                    hts.append(th)
                return qts, hts

            cp_eng = [nc.vector.tensor_copy, nc.scalar.copy]

            def proj_kqq_span(s, dst, w_sb, tiles, sp):
                """one 512-col kT/qq span: single DoubleRow matmul."""
                srow = s * SEGROWS
                k, off = divmod(sp * 512, 1024)
                ps = ps_st.tile([64, 512], f32, tag="st", name="kqp")
                nc.tensor.matmul(
                    ps[:], pair2(w_sb[:, :]),
                    pair2(tiles[k][:, :])[:, :, off:off + 512],
                    start=True, stop=True, perf_mode=DR)
                col = srow + sp * 512
                cp_eng[sp % 2](dst[:, col:col + 512], ps[:])

            def project_part1(s, qts, hts):
                """kT (all spans) + qq spans 0-1: enough for ic=0 scores."""
                for sp in range(4):
                    proj_kqq_span(s, kT_sb, wk_sb, hts, sp)
                for sp in range(2):
                    proj_kqq_span(s, qq_sb, wq_sb, qts, sp)

            def project_part2(s, qts, hts):
                """qq spans 2-3 + V projection."""
                for sp in range(2, 4):
                    proj_kqq_span(s, qq_sb, wq_sb, qts, sp)
                vcp = [nc.vector.tensor_copy, nc.scalar.copy]
                for jt in range(NJT):
                    ps = ps_att.tile([128, HID], f32, tag="att", name="vp")
                    k, off = divmod(jt * 128, 1024)
                    nc.tensor.matmul(
                        ps[:],
                        pair2(hts[k][:, :])[:, :, off:off + 128],
                        pair2(wv_sb[:, :]),
                        start=True, stop=True, perf_mode=DR)
                    base = (s * NJT + jt) * VB
                    vcp[jt % 2](v_sb[:, base:base + HID], ps[:])

            def scores_chunk(s, ic):
                """scores^T (bf16) + exp -> P^T fp8 pair tile."""
                srow = s * SEGROWS
                icol = srow + ic * ICW
                pt_all = pt_pool.tile([128, NJT * ICW], f8, tag="pt")
                for jt in range(NJT):
                    st = ps_st.tile([128, ICW], f32, tag="st")
                    jcol = s * LH + jt * 128
                    for h in range(2):
                        nc.tensor.matmul(
                            st[:, h * 512:(h + 1) * 512],
                            kT_sb[:, jcol:jcol + 128],
                            qq_sb[:, icol + h * 512:icol + (h + 1) * 512],
                            start=True, stop=True)
                    nc.scalar.activation(
                        pt_all[:, jt * ICW:(jt + 1) * ICW], st[:],
                        AF.Exp, scale=SCALE, bias=nb1_sb[:])
                return pt_all

            def attv_pair(s, pt_all, il_a):
                """two interleaved att@V chains (hides ldweights)."""
                atts = [ps_att.tile([128, VB], f32, tag="att",
                                    name=f"att{u}") for u in range(2)]
                for t in range(NJP):
                    vb = (s * NJT + 2 * t) * VB
                    vv = pair2(v_sb[:, vb:vb + 2 * VB])
                    for u in range(2):
                        il = il_a + u
                        nc.tensor.matmul(
                            atts[u][:],
                            pair2(pt_all[:, 2 * t * ICW:(2 * t + 2) * ICW])
                            [:, :, il * 128:(il + 1) * 128],
                            vv,
                            start=(t == 0), stop=(t == NJP - 1),
                            perf_mode=DR)
                return atts

            def x_residual(att, row0, mva0, gi):
                """x0 = att/den + q, plus running LN stats."""
                qt = q_pool.tile([128, HID], bf16, tag="q")
                nc.sync.dma_start(qt[:], q_a[row0:row0 + 128, :])
                rden = ep8_pool.tile([128, 1], f32, tag="rd")
                nc.vector.reciprocal(rden[:], att[:, HID:HID + 1])
                x0 = ep8_pool.tile([128, HID], f32, tag="x0")
                nc.vector.scalar_tensor_tensor(
                    x0[:], att[:, 0:HID], rden[:].opt(), qt[:],
                    op0=Alu.mult, op1=Alu.add)
                mv6 = st8_pool.tile([128, 6], f32, tag="mv6")
                nc.vector.bn_stats(mv6[:], x0[:])
                nc.vector.bn_aggr(mva0[:, 2 * gi:2 * gi + 2], mv6[:])
                return x0

            def ln_rstd(mva, glen, tag):
                ln8 = st8_pool.tile([128, glen], f32, tag=f"ln{tag}")
                nc.scalar.activation(
                    ln8[:].rearrange("p (t o) -> p t o", o=1),
                    mva[:].rearrange("p (t o) -> p t o", o=2)[:, :, 1:2],
                    AF.Ln, bias=eps_sb[:])
                rstd8 = st8_pool.tile([128, glen], f32, tag=f"r{tag}")
                nc.scalar.activation(rstd8[:], ln8[:], AF.Exp, scale=-0.5)
                return rstd8

            def fc_block(x0, mva0, rstd8a, mva1, gi):
                """z = LN0(x0); hres = z@fcw+fb; y0 = relu(hres)+z."""
                z = ep8_pool.tile([128, HID], bf16, tag="z")
                nc.vector.tensor_scalar(
                    z[:], x0[:], mva0[:, 2 * gi:2 * gi + 1].opt(),
                    rstd8a[:, gi:gi + 1].opt(),
                    op0=Alu.subtract, op1=Alu.mult)
                if apply0:
                    z2 = ep_pool.tile([128, HID], bf16, tag="z2")
                    nc.gpsimd.tensor_tensor(z2[:], z[:], n0w_sb[:],
                                            op=Alu.mult)
                    z3 = ep_pool.tile([128, HID], bf16, tag="z3")
                    nc.gpsimd.tensor_tensor(z3[:], z2[:], n0b_sb[:],
                                            op=Alu.add)
                    zf = z3
                else:
                    zf = z
                hres = ps_fc.tile([128, HID], f32, tag="fc")
                nc.tensor.matmul(hres[:], one_sb[:], fb_sb[:],
                                 start=True, stop=False)
                for hh in range(2):
                    tp = ps_tp.tile([128, 128], bf16, tag="tp")
                    nc.tensor.transpose(
                        tp[:], zf[:, hh * 128:(hh + 1) * 128], idt_sb[:])
                    zT = ep_pool.tile([128, 128], bf16, tag=f"zT{hh}")
                    (nc.vector.tensor_copy if hh == 0
                     else nc.scalar.copy)(zT[:], tp[:])
                    nc.tensor.matmul(
                        hres[:], zT[:], fw_sb[:, hh * HID:(hh + 1) * HID],
                        start=False, stop=(hh == 1))
                y0 = ep8_pool.tile([128, HID], f32, tag="y0")
                nc.vector.scalar_tensor_tensor(
                    y0[:], hres[:], 0.0, zf[:], op0=Alu.max, op1=Alu.add)
                mv6b = st8_pool.tile([128, 6], f32, tag="mv6b")
                nc.vector.bn_stats(mv6b[:], y0[:])
                nc.vector.bn_aggr(mva1[:, 2 * gi:2 * gi + 2], mv6b[:])
                return y0

            def store_out(y0, mva1, rstd8b, gi, row0):
                b1 = st8_pool.tile([128, 1], f32, tag="b1")
                nc.vector.tensor_scalar(
                    b1[:], mva1[:, 2 * gi:2 * gi + 1],
                    rstd8b[:, gi:gi + 1].opt(), -1.0,
                    op0=Alu.mult, op1=Alu.mult)
                ot = o_pool.tile([128, HID], f32, tag="ot")
                nc.scalar.activation(
                    ot[:], y0[:], AF.Identity,
                    bias=b1[:], scale=rstd8b[:, gi:gi + 1].opt())
                nc.sync.dma_start(out_a[row0:row0 + 128, :], ot[:])

            def attn_group(s, ic, g0, glen, pt_all):
                srow = s * SEGROWS
                mva0 = st8_pool.tile([128, 2 * glen], f32, tag="mva0")
                xs = []
                for il2 in range(glen // 2):
                    il_a = g0 + 2 * il2
                    atts = attv_pair(s, pt_all, il_a)
                    for u in range(2):
                        il = il_a + u
                        row0 = srow + (ic * NIL + il) * 128
                        xs.append(x_residual(atts[u], row0, mva0,
                                             2 * il2 + u))
                rstd8a = ln_rstd(mva0, glen, "8a")
                mva1 = st8_pool.tile([128, 2 * glen], f32, tag="mva1")
                ys = [fc_block(xs[gi], mva0, rstd8a, mva1, gi)
                      for gi in range(glen)]
                rstd8b = ln_rstd(mva1, glen, "8b")
                for gi in range(glen):
                    row0 = srow + (ic * NIL + g0 + gi) * 128
                    store_out(ys[gi], mva1, rstd8b, gi, row0)

            for s in range(SEGS):
                qts, hts = load_segment(s)
                project_part1(s, qts, hts)
                pt0 = scores_chunk(s, 0)
                project_part2(s, qts, hts)
                last = s == SEGS - 1
                for ic in range(NIC):
                    pt_all = pt0 if ic == 0 else scores_chunk(s, ic)
                    if last and ic == NIC - 1:
                        groups = [(0, 4), (4, 2), (6, 2)]
                    else:
                        groups = [(g * GRP, GRP) for g in range(NIL // GRP)]
                    for g0, glen in groups:
                        attn_group(s, ic, g0, glen, pt_all)

    nc.compile()
    return nc


def _get_nc(apply0: bool):
    key = (bool(apply0),)
    if key not in _built:
        _built[key] = _build(apply0)
    return _built[key]


def _shard(inputs, apply0):
    from concourse import mybir
    bf = mybir.dt.np(mybir.dt.bfloat16)
    f8np = mybir.dt.np(mybir.dt.float8e4)

    q = np.ascontiguousarray(np.asarray(inputs["q"], dtype=np.float32))
    h = np.ascontiguousarray(np.asarray(inputs["h"], dtype=np.float32))
    WQ = np.asarray(inputs["WQ"], dtype=np.float32) * WSC
    WK = np.asarray(inputs["WK"], dtype=np.float32) * WSC
    WV = np.asarray(inputs["WV"], dtype=np.float32) * WSC
    fcw = np.asarray(inputs["fc_w"], dtype=np.float32)
    fcb = np.asarray(inputs["fc_b"], dtype=np.float32)

    WQT = np.ascontiguousarray(WQ.T).astype(f8np)
    WKT = np.ascontiguousarray(WK.T).astype(f8np)
    WVT = np.ascontiguousarray(WV.T).astype(f8np)
    FCWT = np.ascontiguousarray(fcw.T).astype(bf)
    FCB = np.ascontiguousarray(fcb.reshape(1, HID)).astype(bf)
    IDT = np.eye(128, dtype=np.float32).astype(bf)

    in_maps = []
    for c in range(NCORES):
        sl = slice(c * ROWS, (c + 1) * ROWS)
        qc = q[sl]
        m = {
            "qT": np.ascontiguousarray(qc.T).astype(f8np),
            "q": qc.astype(bf),
            "hT": np.ascontiguousarray(h[sl].T).astype(f8np),
            "WQT": WQT, "WKT": WKT, "WVT": WVT,
            "FCWT": FCWT, "FCB": FCB, "IDT": IDT,
        }
        if apply0:
            m["N0W"] = np.ascontiguousarray(
                np.broadcast_to(np.asarray(inputs["norm0_w"], np.float32),
                                (128, HID)))
            m["N0B"] = np.ascontiguousarray(
                np.broadcast_to(np.asarray(inputs["norm0_b"], np.float32),
                                (128, HID)))
        in_maps.append(m)
    return in_maps


def _run(inputs, trace=False, tmpdir=None):
    from concourse import bass_utils

    n0w = np.asarray(inputs["norm0_w"], np.float32)
    n0b = np.asarray(inputs["norm0_b"], np.float32)
    n1w = np.asarray(inputs["norm1_w"], np.float32)
    n1b = np.asarray(inputs["norm1_b"], np.float32)
    apply0 = not (np.allclose(n0w, 1.0) and np.allclose(n0b, 0.0))
    apply1 = not (np.allclose(n1w, 1.0) and np.allclose(n1b, 0.0))

    nc = _get_nc(apply0)
    in_maps = _shard(inputs, apply0)
    res = bass_utils.run_bass_kernel_spmd(
        nc, in_maps, core_ids=list(range(NCORES)), trace=trace,
        tmpdir=tmpdir)
    out = np.concatenate([np.asarray(res.results[c]["out"])
                          for c in range(NCORES)], axis=0)
    if apply1:
        out = out * n1w[None, :] + n1b[None, :]
    return out.astype(np.float32), res


def kernel(**inputs):
    out, _ = _run(inputs, trace=False)
    return out


# revision 67
# speedup vs baseline: 1.2770x; 1.1118x over previous
"""Trainium2 Bass kernel for nn_AttentionBlock (ragged_sequence, 16 equal
segments of 2048 q/kv tokens, HID=256, QD=64) on 8 NeuronCores.

Sharding: 2 segments (4096 rows) per core, weights replicated, outputs
concatenated host-side (attention is block-diagonal per segment -> no
cross-core communication needed).

All attention math (q/k/v projections, scores, probs@V) runs in fp8e4m3
with DoubleRow matmuls (2 k-tiles per pass); FC + layernorms stay
bf16/f32.  Host pre-scales WQ/WK/WV by 8 so fp8 operands sit mid-range;
the score scale and the V "ones column" (=8) cancel it exactly.
"""

import os
import sys

os.environ.setdefault("MYCRO_LOCAL_CACHE", "1")
if "/opt/trn_rl_repo" not in sys.path:
    sys.path.insert(0, "/opt/trn_rl_repo")

import numpy as np

HID = 256
QD = 64
LQ = 2048
LH = 2048
B = 16
NCORES = 8
SEGS = 2                  # segments per core
ROWS = SEGS * LQ          # 4096 q rows per core
EPS = 1e-5
WSC = 8.0                 # host-side WQ/WK/WV pre-scale for fp8 range
SCALE = 1.0 / (8.0 * WSC * WSC)   # 1/sqrt(QD), WQ/WK scales cancelled

_built = {}               # (apply0,) -> nc


def _patch_act_tables():
    """Make the act-table pass choose the combined exp+ln table for every
    activation: blank all other tables (indices preserved so walrus's
    act_func_set_id remap stays correct). Avoids 100+ ACT_TABLE_LOADs
    (1.28us each) from alternating Exp/Ln table picks."""
    import functools
    import concourse.hw_specs as hw_specs
    import concourse.bacc as bacc_mod
    if getattr(hw_specs, "_attn_tables_patched", False):
        return
    orig = hw_specs.get_activation_tables

    @functools.cache
    def patched(arch):
        tabs = dict(orig(arch))
        joint = "natural_log_exp_and_others"
        assert joint in tabs, sorted(tabs)
        return {name: (funcs if name == joint else set())
                for name, funcs in tabs.items()}

    hw_specs.get_activation_tables = patched
    bacc_mod.get_activation_tables = patched
    hw_specs._attn_tables_patched = True


def _build(apply0: bool):
    from concourse import bacc, bass, mybir, tile

    _patch_act_tables()

    dt = mybir.dt
    f32 = dt.float32
    bf16 = dt.bfloat16
    f8 = dt.float8e4
    AF = mybir.ActivationFunctionType
    Alu = mybir.AluOpType
    DR = mybir.MatmulPerfMode.DoubleRow

    NJT = LH // 128           # 16 j-tiles per segment
    NJP = NJT // 2            # 8 j-tile pairs
    NIC = 2                   # 1024-col i-chunks per segment
    ICW = LQ // NIC           # 1024
    NIL = ICW // 128          # 8 i-tiles per chunk
    GRP = 4                   # layernorm stats group (i-tiles)
    VB = HID + 1              # V block width incl ones column
    SEGROWS = ROWS // SEGS    # 2048

    nc = bacc.Bacc("TRN2", target_bir_lowering=False, debug=False,
                   enable_asserts=False)

    qT_d = nc.dram_tensor("qT", [HID, ROWS], f8, kind="ExternalInput")
    q_d = nc.dram_tensor("q", [ROWS, HID], bf16, kind="ExternalInput")
    hT_d = nc.dram_tensor("hT", [HID, ROWS], f8, kind="ExternalInput")
    wqT_d = nc.dram_tensor("WQT", [HID, QD], f8, kind="ExternalInput")
    wkT_d = nc.dram_tensor("WKT", [HID, QD], f8, kind="ExternalInput")
    wvT_d = nc.dram_tensor("WVT", [HID, HID], f8, kind="ExternalInput")
    fwT_d = nc.dram_tensor("FCWT", [HID, HID], bf16, kind="ExternalInput")
    fbb_d = nc.dram_tensor("FCBB", [128, HID], f32, kind="ExternalInput")
    idt_d = nc.dram_tensor("IDT", [128, 128], bf16, kind="ExternalInput")
    if apply0:
        n0w_d = nc.dram_tensor("N0W", [128, HID], f32, kind="ExternalInput")
        n0b_d = nc.dram_tensor("N0B", [128, HID], f32, kind="ExternalInput")
    out_d = nc.dram_tensor("out", [ROWS, HID], f32, kind="ExternalOutput")

    qT_a, q_a, hT_a = qT_d.ap(), q_d.ap(), hT_d.ap()
    out_a = out_d.ap()

    def pair2(ap2d):
        """[p, 2*W] -> [p, 2, W] view for DoubleRow operands."""
        return ap2d.rearrange("p (two w) -> p two w", two=2)

    with tile.TileContext(nc) as tc:
        with (
            tc.tile_pool(name="const", bufs=1) as cpool,
            tc.tile_pool(name="kqq", bufs=1) as kqq_pool,
            tc.tile_pool(name="vsb", bufs=1) as v_pool,
            tc.tile_pool(name="qhT", bufs=2) as qh_pool,
            tc.tile_pool(name="pt", bufs=3) as pt_pool,
            tc.tile_pool(name="qrow", bufs=4) as q_pool,
            tc.tile_pool(name="ep", bufs=6) as ep_pool,
            tc.tile_pool(name="ep8", bufs=10) as ep8_pool,
            tc.tile_pool(name="st8", bufs=12) as st8_pool,
            tc.tile_pool(name="outp", bufs=4) as o_pool,
            tc.tile_pool(name="ps_st", bufs=2,
                         space=bass.MemorySpace.PSUM) as ps_st,
            tc.tile_pool(name="ps_att", bufs=2,
                         space=bass.MemorySpace.PSUM) as ps_att,
            tc.tile_pool(name="ps_fc", bufs=1,
                         space=bass.MemorySpace.PSUM) as ps_fc,
            tc.tile_pool(name="ps_tp", bufs=1,
                         space=bass.MemorySpace.PSUM) as ps_tp,
        ):
            # ---- constants ----
            wq_sb = cpool.tile([128, 2 * QD], f8)      # [e, (chunk, c)]
            wk_sb = cpool.tile([128, 2 * QD], f8)
            wv_sb = cpool.tile([128, 2 * HID], f8)     # [e, (chunk, d)]
            fw_sb = cpool.tile([128, 2 * HID], bf16)   # fc_w.T chunks
            fbb_sb = cpool.tile([128, HID], f32)       # fc_b broadcast
            nc.sync.dma_start(fbb_sb[:], fbb_d.ap()[:, :])
            idt_sb = cpool.tile([128, 128], bf16)
            nc.sync.dma_start(idt_sb[:], idt_d.ap()[:, :])
            for e in range(2):
                nc.sync.dma_start(wq_sb[:, e * QD:(e + 1) * QD],
                                  wqT_d.ap()[e * 128:(e + 1) * 128, :])
                nc.sync.dma_start(wk_sb[:, e * QD:(e + 1) * QD],
                                  wkT_d.ap()[e * 128:(e + 1) * 128, :])
                nc.sync.dma_start(wv_sb[:, e * HID:(e + 1) * HID],
                                  wvT_d.ap()[e * 128:(e + 1) * 128, :])
                nc.sync.dma_start(fw_sb[:, e * HID:(e + 1) * HID],
                                  fwT_d.ap()[e * 128:(e + 1) * 128, :])

            eps_sb = cpool.tile([128, 1], f32)
            nc.vector.memset(eps_sb[:], EPS)
            # exp bias: p = exp(score*scale - 1) centers p in fp8e4m3 range
            nb1_sb = cpool.tile([128, 1], f32)
            nc.vector.memset(nb1_sb[:], -1.0)
            if apply0:
                n0w_sb = cpool.tile([128, HID], f32)
                n0b_sb = cpool.tile([128, HID], f32)
                nc.sync.dma_start(n0w_sb[:], n0w_d.ap()[:, :])
                nc.sync.dma_start(n0b_sb[:], n0b_d.ap()[:, :])

            # persistent activations
            kT_sb = kqq_pool.tile([64, ROWS], f8)       # K^T  [c, j]
            qq_sb = kqq_pool.tile([64, ROWS], f8)       # qq^T [c, i]
            v_sb = v_pool.tile([128, SEGS * NJT * VB], f8)
            # every V block's ones column, set once
            nc.gpsimd.memset(
                v_sb[:, :].rearrange("p (b w) -> p b w", w=VB)
                [:, :, HID:HID + 1], WSC)

            def load_segment(s):
                """DMA qT/hT (fp8) tiles for segment s."""
                srow = s * SEGROWS
                qts, hts = [], []
                for k in range(2):          # 1024-col spans
                    span = slice(srow + k * 1024, srow + (k + 1) * 1024)
                    tq = qh_pool.tile([128, 2048], f8, tag=f"qt{k}")
                    th = qh_pool.tile([128, 2048], f8, tag=f"ht{k}")
                    for e in range(2):
                        nc.sync.dma_start(
                            tq[:, e * 1024:(e + 1) * 1024],
                            qT_a[e * 128:(e + 1) * 128, span])
                        nc.sync.dma_start(
                            th[:, e * 1024:(e + 1) * 1024],
                            hT_a[e * 128:(e + 1) * 128, span])
                    qts.append(tq)
                    hts.append(th)
                return qts, hts

            cp_eng = [nc.scalar.copy, nc.scalar.copy]

            def proj_kqq_span(s, dst, w_sb, tiles, sp):
                """one 512-col kT/qq span: single DoubleRow matmul."""
                srow = s * SEGROWS
                k, off = divmod(sp * 512, 1024)
                ps = ps_st.tile([64, 512], f32, tag="st", name="kqp")
                nc.tensor.matmul(
                    ps[:], pair2(w_sb[:, :]),
                    pair2(tiles[k][:, :])[:, :, off:off + 512],
                    start=True, stop=True, perf_mode=DR)
                col = srow + sp * 512
                cp_eng[sp % 2](dst[:, col:col + 512], ps[:])

            def project_part1(s, qts, hts):
                """kT (all spans) + qq spans 0-1: enough for ic=0 scores."""
                for sp in range(4):
                    proj_kqq_span(s, kT_sb, wk_sb, hts, sp)
                for sp in range(2):
                    proj_kqq_span(s, qq_sb, wq_sb, qts, sp)

            def project_part2(s, qts, hts):
                """qq spans 2-3 + V projection."""
                for sp in range(2, 4):
                    proj_kqq_span(s, qq_sb, wq_sb, qts, sp)
                vcp = [nc.vector.tensor_copy, nc.vector.tensor_copy]
                for jt in range(NJT):
                    ps = ps_att.tile([128, HID], f32, tag="att", name="vp")
                    k, off = divmod(jt * 128, 1024)
                    nc.tensor.matmul(
                        ps[:],
                        pair2(hts[k][:, :])[:, :, off:off + 128],
                        pair2(wv_sb[:, :]),
                        start=True, stop=True, perf_mode=DR)
                    base = (s * NJT + jt) * VB
                    vcp[jt % 2](v_sb[:, base:base + HID], ps[:])

            def scores_chunk(s, ic):
                """scores^T (bf16) + exp -> P^T fp8 pair tile."""
                srow = s * SEGROWS
                icol = srow + ic * ICW
                pt_all = pt_pool.tile([128, NJT * ICW], f8, tag="pt")
                for jt in range(NJT):
                    st = ps_st.tile([128, ICW], f32, tag="st")
                    jcol = s * LH + jt * 128
                    for h in range(2):
                        nc.tensor.matmul(
                            st[:, h * 512:(h + 1) * 512],
                            kT_sb[:, jcol:jcol + 128],
                            qq_sb[:, icol + h * 512:icol + (h + 1) * 512],
                            start=True, stop=True)
                    nc.scalar.activation(
                        pt_all[:, jt * ICW:(jt + 1) * ICW], st[:],
                        AF.Exp, scale=SCALE, bias=nb1_sb[:])
                return pt_all

            def attv_pair(s, pt_all, il_a):
                """two interleaved att@V chains (hides ldweights)."""
                atts = [ps_att.tile([128, VB], f32, tag="att",
                                    name=f"att{u}") for u in range(2)]
                for t in range(NJP):
                    vb = (s * NJT + 2 * t) * VB
                    vv = pair2(v_sb[:, vb:vb + 2 * VB])
                    for u in range(2):
                        il = il_a + u
                        nc.tensor.matmul(
                            atts[u][:],
                            pair2(pt_all[:, 2 * t * ICW:(2 * t + 2) * ICW])
                            [:, :, il * 128:(il + 1) * 128],
                            vv,
                            start=(t == 0), stop=(t == NJP - 1),
                            perf_mode=DR)
                return atts

            def x_residual(att, row0, mva0, gi):
                """x0 = att/den + q (bf16: unlocks DVE 2x modes), + stats."""
                qt = q_pool.tile([128, HID], bf16, tag="q")
                nc.sync.dma_start(qt[:], q_a[row0:row0 + 128, :])
                rden = ep8_pool.tile([128, 1], f32, tag="rd")
                nc.vector.reciprocal(rden[:], att[:, HID:HID + 1])
                x0 = ep8_pool.tile([128, HID], bf16, tag="x0")
                nc.vector.scalar_tensor_tensor(
                    x0[:], att[:, 0:HID], rden[:].opt(), qt[:],
                    op0=Alu.mult, op1=Alu.add)
                mv6 = st8_pool.tile([128, 6], f32, tag="mv6")
                nc.vector.bn_stats(mv6[:], x0[:])
                nc.vector.bn_aggr(mva0[:, 2 * gi:2 * gi + 2], mv6[:])
                return x0

            def ln_rstd(mva, glen, tag):
                ln8 = st8_pool.tile([128, glen], f32, tag=f"ln{tag}")
                nc.scalar.activation(
                    ln8[:].rearrange("p (t o) -> p t o", o=1),
                    mva[:].rearrange("p (t o) -> p t o", o=2)[:, :, 1:2],
                    AF.Ln, bias=eps_sb[:])
                rstd8 = st8_pool.tile([128, glen], f32, tag=f"r{tag}")
                nc.scalar.activation(rstd8[:], ln8[:], AF.Exp, scale=-0.5)
                return rstd8

            def fc_block(x0, mva0, rstd8a, mva1, gi):
                """z = LN0(x0); hres = z@fcw+fb; y0 = relu(hres)+z."""
                z = ep8_pool.tile([128, HID], bf16, tag="z")
                nc.vector.tensor_scalar(
                    z[:], x0[:], mva0[:, 2 * gi:2 * gi + 1].opt(),
                    rstd8a[:, gi:gi + 1].opt(),
                    op0=Alu.subtract, op1=Alu.mult)
                if apply0:
                    z2 = ep_pool.tile([128, HID], bf16, tag="z2")
                    nc.gpsimd.tensor_tensor(z2[:], z[:], n0w_sb[:],
                                            op=Alu.mult)
                    z3 = ep_pool.tile([128, HID], bf16, tag="z3")
                    nc.gpsimd.tensor_tensor(z3[:], z2[:], n0b_sb[:],
                                            op=Alu.add)
                    zf = z3
                else:
                    zf = z
                hres = ps_fc.tile([128, HID], f32, tag="fc")
                nc.vector.tensor_copy(hres[:], fbb_sb[:])  # bias prefill
                for hh in range(2):
                    tp = ps_tp.tile([128, 128], bf16, tag="tp")
                    nc.tensor.transpose(
                        tp[:], zf[:, hh * 128:(hh + 1) * 128], idt_sb[:])
                    zT = ep_pool.tile([128, 128], bf16, tag=f"zT{hh}")
                    nc.vector.tensor_copy(zT[:], tp[:])
                    nc.tensor.matmul(
                        hres[:], zT[:], fw_sb[:, hh * HID:(hh + 1) * HID],
                        start=False, stop=(hh == 1))
                y0 = ep8_pool.tile([128, HID], bf16, tag="y0")
                nc.vector.scalar_tensor_tensor(
                    y0[:], hres[:], 0.0, zf[:], op0=Alu.max, op1=Alu.add)
                mv6b = st8_pool.tile([128, 6], f32, tag="mv6b")
                nc.vector.bn_stats(mv6b[:], y0[:])
                nc.vector.bn_aggr(mva1[:, 2 * gi:2 * gi + 2], mv6b[:])
                return y0

            def store_out(y0, mva1, rstd8b, gi, row0):
                b1 = st8_pool.tile([128, 1], f32, tag="b1")
                nc.vector.tensor_scalar(
                    b1[:], mva1[:, 2 * gi:2 * gi + 1],
                    rstd8b[:, gi:gi + 1].opt(), -1.0,
                    op0=Alu.mult, op1=Alu.mult)
                ot = o_pool.tile([128, HID], f32, tag="ot")
                if gi % 2 == 0:
                    nc.scalar.activation(
                        ot[:], y0[:], AF.Identity,
                        bias=b1[:], scale=rstd8b[:, gi:gi + 1].opt())
                else:
                    nc.vector.tensor_scalar(
                        ot[:], y0[:], rstd8b[:, gi:gi + 1].opt(),
                        b1[:].opt(), op0=Alu.mult, op1=Alu.add)
                nc.sync.dma_start(out_a[row0:row0 + 128, :], ot[:])

            def attn_group(s, ic, g0, glen, pt_all):
                srow = s * SEGROWS
                mva0 = st8_pool.tile([128, 2 * glen], f32, tag="mva0")
                xs = []
                for il2 in range(glen // 2):
                    il_a = g0 + 2 * il2
                    atts = attv_pair(s, pt_all, il_a)
                    for u in range(2):
                        il = il_a + u
                        row0 = srow + (ic * NIL + il) * 128
                        xs.append(x_residual(atts[u], row0, mva0,
                                             2 * il2 + u))
                rstd8a = ln_rstd(mva0, glen, "8a")
                mva1 = st8_pool.tile([128, 2 * glen], f32, tag="mva1")
                ys = [fc_block(xs[gi], mva0, rstd8a, mva1, gi)
                      for gi in range(glen)]
                rstd8b = ln_rstd(mva1, glen, "8b")
                for gi in range(glen):
                    row0 = srow + (ic * NIL + g0 + gi) * 128
                    store_out(ys[gi], mva1, rstd8b, gi, row0)

            for s in range(SEGS):
                qts, hts = load_segment(s)
                project_part1(s, qts, hts)
                pt0 = scores_chunk(s, 0)
                project_part2(s, qts, hts)
                last = s == SEGS - 1
                for ic in range(NIC):
                    pt_all = pt0 if ic == 0 else scores_chunk(s, ic)
                    if last and ic == NIC - 1:
                        groups = [(0, 4), (4, 2), (6, 2)]
                    else:
                        groups = [(g * GRP, GRP) for g in range(NIL // GRP)]
                    for g0, glen in groups:
                        attn_group(s, ic, g0, glen, pt_all)

    nc.compile()
    return nc


def _get_nc(apply0: bool):
    key = (bool(apply0),)
    if key not in _built:
        _built[key] = _build(apply0)
    return _built[key]


def _shard(inputs, apply0):
    from concourse import mybir
    bf = mybir.dt.np(mybir.dt.bfloat16)
    f8np = mybir.dt.np(mybir.dt.float8e4)

    q = np.ascontiguousarray(np.asarray(inputs["q"], dtype=np.float32))
    h = np.ascontiguousarray(np.asarray(inputs["h"], dtype=np.float32))
    WQ = np.asarray(inputs["WQ"], dtype=np.float32) * WSC
    WK = np.asarray(inputs["WK"], dtype=np.float32) * WSC
    WV = np.asarray(inputs["WV"], dtype=np.float32) * WSC
    fcw = np.asarray(inputs["fc_w"], dtype=np.float32)
    fcb = np.asarray(inputs["fc_b"], dtype=np.float32)

    WQT = np.ascontiguousarray(WQ.T).astype(f8np)
    WKT = np.ascontiguousarray(WK.T).astype(f8np)
    WVT = np.ascontiguousarray(WV.T).astype(f8np)
    FCWT = np.ascontiguousarray(fcw.T).astype(bf)
    FCBB = np.ascontiguousarray(
        np.broadcast_to(fcb.reshape(1, HID), (128, HID))).astype(np.float32)
    IDT = np.eye(128, dtype=np.float32).astype(bf)

    in_maps = []
    for c in range(NCORES):
        sl = slice(c * ROWS, (c + 1) * ROWS)
        qc = q[sl]
        m = {
            "qT": np.ascontiguousarray(qc.T).astype(f8np),
            "q": qc.astype(bf),
            "hT": np.ascontiguousarray(h[sl].T).astype(f8np),
            "WQT": WQT, "WKT": WKT, "WVT": WVT,
            "FCWT": FCWT, "FCBB": FCBB, "IDT": IDT,
        }
        if apply0:
            m["N0W"] = np.ascontiguousarray(
                np.broadcast_to(np.asarray(inputs["norm0_w"], np.float32),
                                (128, HID)))
            m["N0B"] = np.ascontiguousarray(
                np.broadcast_to(np.asarray(inputs["norm0_b"], np.float32),
                                (128, HID)))
        in_maps.append(m)
    return in_maps


def _run(inputs, trace=False, tmpdir=None):
    from concourse import bass_utils

    n0w = np.asarray(inputs["norm0_w"], np.float32)
    n0b = np.asarray(inputs["norm0_b"], np.float32)
    n1w = np.asarray(inputs["norm1_w"], np.float32)
    n1b = np.asarray(inputs["norm1_b"], np.float32)
    apply0 = not (np.allclose(n0w, 1.0) and np.allclose(n0b, 0.0))
    apply1 = not (np.allclose(n1w, 1.0) and np.allclose(n1b, 0.0))

    nc = _get_nc(apply0)
    in_maps = _shard(inputs, apply0)
    res = bass_utils.run_bass_kernel_spmd(
        nc, in_maps, core_ids=list(range(NCORES)), trace=trace,
        tmpdir=tmpdir)
    out = np.concatenate([np.asarray(res.results[c]["out"])
                          for c in range(NCORES)], axis=0)
    if apply1:
        out = out * n1w[None, :] + n1b[None, :]
    return out.astype(np.float32), res


def kernel(**inputs):
    out, _ = _run(inputs, trace=False)
    return out


# revision 72
# speedup vs baseline: 1.2922x; 1.0119x over previous
"""Trainium2 Bass kernel for nn_AttentionBlock (ragged_sequence, 16 equal
segments of 2048 q/kv tokens, HID=256, QD=64) on 8 NeuronCores.

Sharding: 2 segments (4096 rows) per core, weights replicated, outputs
concatenated host-side (attention is block-diagonal per segment -> no
cross-core communication needed).

All attention math (q/k/v projections, scores, probs@V) runs in fp8e4m3
with DoubleRow matmuls (2 k-tiles per pass); FC + layernorms stay
bf16/f32.  Host pre-scales WQ/WK/WV by 8 so fp8 operands sit mid-range;
the score scale and the V "ones column" (=8) cancel it exactly.
"""

import os
import sys

os.environ.setdefault("MYCRO_LOCAL_CACHE", "1")
if "/opt/trn_rl_repo" not in sys.path:
    sys.path.insert(0, "/opt/trn_rl_repo")

import numpy as np

HID = 256
QD = 64
LQ = 2048
LH = 2048
B = 16
NCORES = 8
SEGS = 2                  # segments per core
ROWS = SEGS * LQ          # 4096 q rows per core
EPS = 1e-5
WSC = 8.0                 # host-side WQ/WK/WV pre-scale for fp8 range
SCALE = 1.0 / (8.0 * WSC * WSC)   # 1/sqrt(QD), WQ/WK scales cancelled

_built = {}               # (apply0,) -> nc


def _patch_act_tables():
    """Make the act-table pass choose the combined exp+ln table for every
    activation: blank all other tables (indices preserved so walrus's
    act_func_set_id remap stays correct). Avoids 100+ ACT_TABLE_LOADs
    (1.28us each) from alternating Exp/Ln table picks."""
    import functools
    import concourse.hw_specs as hw_specs
    import concourse.bacc as bacc_mod
    if getattr(hw_specs, "_attn_tables_patched", False):
        return
    orig = hw_specs.get_activation_tables

    @functools.cache
    def patched(arch):
        tabs = dict(orig(arch))
        joint = "natural_log_exp_and_others"
        assert joint in tabs, sorted(tabs)
        return {name: (funcs if name == joint else set())
                for name, funcs in tabs.items()}

    hw_specs.get_activation_tables = patched
    bacc_mod.get_activation_tables = patched
    hw_specs._attn_tables_patched = True


def _build(apply0: bool):
    from concourse import bacc, bass, mybir, tile

    _patch_act_tables()

    dt = mybir.dt
    f32 = dt.float32
    bf16 = dt.bfloat16
    f8 = dt.float8e4
    AF = mybir.ActivationFunctionType
    Alu = mybir.AluOpType
    DR = mybir.MatmulPerfMode.DoubleRow

    NJT = LH // 128           # 16 j-tiles per segment
    NJP = NJT // 2            # 8 j-tile pairs
    NIC = 2                   # 1024-col i-chunks per segment
    ICW = LQ // NIC           # 1024
    NIL = ICW // 128          # 8 i-tiles per chunk
    GRP = 4                   # layernorm stats group (i-tiles)
    VB = HID + 1              # V block width incl ones column
    SEGROWS = ROWS // SEGS    # 2048

    nc = bacc.Bacc("TRN2", target_bir_lowering=False, debug=False,
                   enable_asserts=False)

    # qT/hT in pre-paired layout [128, (e_chunk, col)]: one DMA per segment
    qT_d = nc.dram_tensor("qTP", [128, 2 * ROWS], f8, kind="ExternalInput")
    q_d = nc.dram_tensor("q", [ROWS, HID], bf16, kind="ExternalInput")
    hT_d = nc.dram_tensor("hTP", [128, 2 * ROWS], f8, kind="ExternalInput")
    wqT_d = nc.dram_tensor("WQT", [HID, QD], f8, kind="ExternalInput")
    wkT_d = nc.dram_tensor("WKT", [HID, QD], f8, kind="ExternalInput")
    wvT_d = nc.dram_tensor("WVT", [HID, HID], f8, kind="ExternalInput")
    fwT_d = nc.dram_tensor("FCWT", [HID, HID], bf16, kind="ExternalInput")
    fbb_d = nc.dram_tensor("FCBB", [128, HID], f32, kind="ExternalInput")
    idt_d = nc.dram_tensor("IDT", [128, 128], bf16, kind="ExternalInput")
    if apply0:
        n0w_d = nc.dram_tensor("N0W", [128, HID], f32, kind="ExternalInput")
        n0b_d = nc.dram_tensor("N0B", [128, HID], f32, kind="ExternalInput")
    out_d = nc.dram_tensor("out", [ROWS, HID], f32, kind="ExternalOutput")

    qT_a, q_a, hT_a = qT_d.ap(), q_d.ap(), hT_d.ap()
    out_a = out_d.ap()

    def pair2(ap2d):
        """[p, 2*W] -> [p, 2, W] view for DoubleRow operands."""
        return ap2d.rearrange("p (two w) -> p two w", two=2)

    with tile.TileContext(nc) as tc:
        with (
            tc.tile_pool(name="const", bufs=1) as cpool,
            tc.tile_pool(name="kqq", bufs=1) as kqq_pool,
            tc.tile_pool(name="vsb", bufs=1) as v_pool,
            tc.tile_pool(name="qhT", bufs=2) as qh_pool,
            tc.tile_pool(name="pt", bufs=3) as pt_pool,
            tc.tile_pool(name="qrow", bufs=4) as q_pool,
            tc.tile_pool(name="ep", bufs=6) as ep_pool,
            tc.tile_pool(name="ep8", bufs=10) as ep8_pool,
            tc.tile_pool(name="st8", bufs=12) as st8_pool,
            tc.tile_pool(name="outp", bufs=4) as o_pool,
            tc.tile_pool(name="ps_st", bufs=2,
                         space=bass.MemorySpace.PSUM) as ps_st,
            tc.tile_pool(name="ps_att", bufs=2,
                         space=bass.MemorySpace.PSUM) as ps_att,
            tc.tile_pool(name="ps_fc", bufs=1,
                         space=bass.MemorySpace.PSUM) as ps_fc,
            tc.tile_pool(name="ps_tp", bufs=1,
                         space=bass.MemorySpace.PSUM) as ps_tp,
        ):
            # ---- constants ----
            wq_sb = cpool.tile([128, 2 * QD], f8)      # [e, (chunk, c)]
            wk_sb = cpool.tile([128, 2 * QD], f8)
            wv_sb = cpool.tile([128, 2 * HID], f8)     # [e, (chunk, d)]
            fw_sb = cpool.tile([128, 2 * HID], bf16)   # fc_w.T chunks
            fbb_sb = cpool.tile([128, HID], f32)       # fc_b broadcast
            nc.sync.dma_start(fbb_sb[:], fbb_d.ap()[:, :])
            idt_sb = cpool.tile([128, 128], bf16)
            nc.sync.dma_start(idt_sb[:], idt_d.ap()[:, :])
            for e in range(2):
                nc.sync.dma_start(wq_sb[:, e * QD:(e + 1) * QD],
                                  wqT_d.ap()[e * 128:(e + 1) * 128, :])
                nc.sync.dma_start(wk_sb[:, e * QD:(e + 1) * QD],
                                  wkT_d.ap()[e * 128:(e + 1) * 128, :])
                nc.sync.dma_start(wv_sb[:, e * HID:(e + 1) * HID],
                                  wvT_d.ap()[e * 128:(e + 1) * 128, :])
                nc.sync.dma_start(fw_sb[:, e * HID:(e + 1) * HID],
                                  fwT_d.ap()[e * 128:(e + 1) * 128, :])

            eps_sb = cpool.tile([128, 1], f32)
            nc.vector.memset(eps_sb[:], EPS)
            # exp bias: p = exp(score*scale - 1) centers p in fp8e4m3 range
            nb1_sb = cpool.tile([128, 1], f32)
            nc.vector.memset(nb1_sb[:], -1.0)
            if apply0:
                n0w_sb = cpool.tile([128, HID], f32)
                n0b_sb = cpool.tile([128, HID], f32)
                nc.sync.dma_start(n0w_sb[:], n0w_d.ap()[:, :])
                nc.sync.dma_start(n0b_sb[:], n0b_d.ap()[:, :])

            # persistent activations
            kT_sb = kqq_pool.tile([64, ROWS], f8)       # K^T  [c, j]
            qq_sb = kqq_pool.tile([64, ROWS], f8)       # qq^T [c, i]
            v_sb = v_pool.tile([128, SEGS * NJT * VB], f8)
            # every V block's ones column, set once
            nc.gpsimd.memset(
                v_sb[:, :].rearrange("p (b w) -> p b w", w=VB)
                [:, :, HID:HID + 1], WSC)

            def load_segment(s):
                """one big paired DMA per tensor per segment, on separate
                hwdge queues (sync for hT, scalar for qT)."""
                srow = s * SEGROWS
                th = qh_pool.tile([128, 2 * SEGROWS], f8, tag="ht")
                tq = qh_pool.tile([128, 2 * SEGROWS], f8, tag="qt")
                nc.sync.dma_start(
                    pair2(th[:, :]),
                    pair2(hT_a[:, :])[:, :, srow:srow + SEGROWS])
                nc.scalar.dma_start(
                    pair2(tq[:, :]),
                    pair2(qT_a[:, :])[:, :, srow:srow + SEGROWS])
                return tq, th

            cp_eng = [nc.vector.tensor_copy, nc.scalar.copy]

            def proj_kqq_span(s, dst, w_sb, t, sp):
                """one 512-col kT/qq span: single DoubleRow matmul."""
                srow = s * SEGROWS
                off = sp * 512
                ps = ps_st.tile([64, 512], f32, tag="st", name="kqp")
                nc.tensor.matmul(
                    ps[:], pair2(w_sb[:, :]),
                    pair2(t[:, :])[:, :, off:off + 512],
                    start=True, stop=True, perf_mode=DR)
                col = srow + off
                cp_eng[sp % 2](dst[:, col:col + 512], ps[:])

            def project_part1(s, tq, th):
                """kT + qq spans 0-1 first: unblocks ic=0 scores fast."""
                proj_kqq_span(s, kT_sb, wk_sb, th, 0)
                proj_kqq_span(s, qq_sb, wq_sb, tq, 0)
                proj_kqq_span(s, qq_sb, wq_sb, tq, 1)
                for sp in range(1, 4):
                    proj_kqq_span(s, kT_sb, wk_sb, th, sp)

            def project_part2(s, tq, th):
                """qq spans 2-3 + V projection."""
                for sp in range(2, 4):
                    proj_kqq_span(s, qq_sb, wq_sb, tq, sp)
                for jt in range(NJT):
                    ps = ps_att.tile([128, HID], f32, tag="att", name="vp")
                    off = jt * 128
                    nc.tensor.matmul(
                        ps[:],
                        pair2(th[:, :])[:, :, off:off + 128],
                        pair2(wv_sb[:, :]),
                        start=True, stop=True, perf_mode=DR)
                    base = (s * NJT + jt) * VB
                    nc.vector.tensor_copy(v_sb[:, base:base + HID], ps[:])

            def scores_chunk(s, ic):
                """scores^T (bf16) + exp -> P^T fp8 pair tile."""
                srow = s * SEGROWS
                icol = srow + ic * ICW
                pt_all = pt_pool.tile([128, NJT * ICW], f8, tag="pt")
                for jt in range(NJT):
                    st = ps_st.tile([128, ICW], f32, tag="st")
                    jcol = s * LH + jt * 128
                    for h in range(2):
                        nc.tensor.matmul(
                            st[:, h * 512:(h + 1) * 512],
                            kT_sb[:, jcol:jcol + 128],
                            qq_sb[:, icol + h * 512:icol + (h + 1) * 512],
                            start=True, stop=True)
                    nc.scalar.activation(
                        pt_all[:, jt * ICW:(jt + 1) * ICW], st[:],
                        AF.Exp, scale=SCALE, bias=nb1_sb[:])
                return pt_all

            def attv_pair(s, pt_all, il_a):
                """two interleaved att@V chains (hides ldweights)."""
                atts = [ps_att.tile([128, VB], f32, tag="att",
                                    name=f"att{u}") for u in range(2)]
                for t in range(NJP):
                    vb = (s * NJT + 2 * t) * VB
                    vv = pair2(v_sb[:, vb:vb + 2 * VB])
                    for u in range(2):
                        il = il_a + u
                        nc.tensor.matmul(
                            atts[u][:],
                            pair2(pt_all[:, 2 * t * ICW:(2 * t + 2) * ICW])
                            [:, :, il * 128:(il + 1) * 128],
                            vv,
                            start=(t == 0), stop=(t == NJP - 1),
                            perf_mode=DR)
                return atts

            def x_residual(att, row0, mva0, gi):
                """x0 = att/den + q (bf16: unlocks DVE 2x modes), + stats."""
                qt = q_pool.tile([128, HID], bf16, tag="q")
                nc.sync.dma_start(qt[:], q_a[row0:row0 + 128, :])
                rden = ep8_pool.tile([128, 1], f32, tag="rd")
                nc.vector.reciprocal(rden[:], att[:, HID:HID + 1])
                x0 = ep8_pool.tile([128, HID], bf16, tag="x0")
                nc.vector.scalar_tensor_tensor(
                    x0[:], att[:, 0:HID], rden[:].opt(), qt[:],
                    op0=Alu.mult, op1=Alu.add)
                mv6 = st8_pool.tile([128, 6], f32, tag="mv6")
                nc.vector.bn_stats(mv6[:], x0[:])
                nc.vector.bn_aggr(mva0[:, 2 * gi:2 * gi + 2], mv6[:])
                return x0

            def ln_rstd(mva, glen, tag):
                ln8 = st8_pool.tile([128, glen], f32, tag=f"ln{tag}")
                nc.scalar.activation(
                    ln8[:].rearrange("p (t o) -> p t o", o=1),
                    mva[:].rearrange("p (t o) -> p t o", o=2)[:, :, 1:2],
                    AF.Ln, bias=eps_sb[:])
                rstd8 = st8_pool.tile([128, glen], f32, tag=f"r{tag}")
                nc.scalar.activation(rstd8[:], ln8[:], AF.Exp, scale=-0.5)
                return rstd8

            def fc_block(x0, mva0, rstd8a, mva1, gi):
                """z = LN0(x0); hres = z@fcw+fb; y0 = relu(hres)+z."""
                z = ep8_pool.tile([128, HID], bf16, tag="z")
                nc.vector.tensor_scalar(
                    z[:], x0[:], mva0[:, 2 * gi:2 * gi + 1].opt(),
                    rstd8a[:, gi:gi + 1].opt(),
                    op0=Alu.subtract, op1=Alu.mult)
                if apply0:
                    z2 = ep_pool.tile([128, HID], bf16, tag="z2")
                    nc.gpsimd.tensor_tensor(z2[:], z[:], n0w_sb[:],
                                            op=Alu.mult)
                    z3 = ep_pool.tile([128, HID], bf16, tag="z3")
                    nc.gpsimd.tensor_tensor(z3[:], z2[:], n0b_sb[:],
                                            op=Alu.add)
                    zf = z3
                else:
                    zf = z
                hres = ps_fc.tile([128, HID], f32, tag="fc")
                nc.vector.tensor_copy(hres[:], fbb_sb[:])  # bias prefill
                for hh in range(2):
                    tp = ps_tp.tile([128, 128], bf16, tag="tp")
                    nc.tensor.transpose(
                        tp[:], zf[:, hh * 128:(hh + 1) * 128], idt_sb[:])
                    zT = ep_pool.tile([128, 128], bf16, tag=f"zT{hh}")
                    nc.vector.tensor_copy(zT[:], tp[:])
                    nc.tensor.matmul(
                        hres[:], zT[:], fw_sb[:, hh * HID:(hh + 1) * HID],
                        start=False, stop=(hh == 1))
                y0 = ep8_pool.tile([128, HID], bf16, tag="y0")
                nc.vector.scalar_tensor_tensor(
                    y0[:], hres[:], 0.0, zf[:], op0=Alu.max, op1=Alu.add)
                mv6b = st8_pool.tile([128, 6], f32, tag="mv6b")
                nc.vector.bn_stats(mv6b[:], y0[:])
                nc.vector.bn_aggr(mva1[:, 2 * gi:2 * gi + 2], mv6b[:])
                return y0

            def store_out(y0, mva1, rstd8b, gi, row0):
                b1 = st8_pool.tile([128, 1], f32, tag="b1")
                nc.vector.tensor_scalar(
                    b1[:], mva1[:, 2 * gi:2 * gi + 1],
                    rstd8b[:, gi:gi + 1].opt(), -1.0,
                    op0=Alu.mult, op1=Alu.mult)
                ot = o_pool.tile([128, HID], f32, tag="ot")
                if gi % 2 == 0:
                    nc.scalar.activation(
                        ot[:], y0[:], AF.Identity,
                        bias=b1[:], scale=rstd8b[:, gi:gi + 1].opt())
                else:
                    nc.vector.tensor_scalar(
                        ot[:], y0[:], rstd8b[:, gi:gi + 1].opt(),
                        b1[:].opt(), op0=Alu.mult, op1=Alu.add)
                nc.sync.dma_start(out_a[row0:row0 + 128, :], ot[:])

            def attn_group(s, ic, g0, glen, pt_all):
                srow = s * SEGROWS
                mva0 = st8_pool.tile([128, 2 * glen], f32, tag="mva0")
                xs = []
                for il2 in range(glen // 2):
                    il_a = g0 + 2 * il2
                    atts = attv_pair(s, pt_all, il_a)
                    for u in range(2):
                        il = il_a + u
                        row0 = srow + (ic * NIL + il) * 128
                        xs.append(x_residual(atts[u], row0, mva0,
                                             2 * il2 + u))
                rstd8a = ln_rstd(mva0, glen, "8a")
                mva1 = st8_pool.tile([128, 2 * glen], f32, tag="mva1")
                ys = [fc_block(xs[gi], mva0, rstd8a, mva1, gi)
                      for gi in range(glen)]
                rstd8b = ln_rstd(mva1, glen, "8b")
                for gi in range(glen):
                    row0 = srow + (ic * NIL + g0 + gi) * 128
                    store_out(ys[gi], mva1, rstd8b, gi, row0)

            for s in range(SEGS):
                tq, th = load_segment(s)
                project_part1(s, tq, th)
                pt0 = scores_chunk(s, 0)
                project_part2(s, tq, th)
                for ic in range(NIC):
                    pt_all = pt0 if ic == 0 else scores_chunk(s, ic)
                    for g in range(NIL // GRP):
                        attn_group(s, ic, g * GRP, GRP, pt_all)

    nc.compile()
    return nc


def _get_nc(apply0: bool):
    key = (bool(apply0),)
    if key not in _built:
        _built[key] = _build(apply0)
    return _built[key]


def _shard(inputs, apply0):
    from concourse import mybir
    bf = mybir.dt.np(mybir.dt.bfloat16)
    f8np = mybir.dt.np(mybir.dt.float8e4)

    q = np.ascontiguousarray(np.asarray(inputs["q"], dtype=np.float32))
    h = np.ascontiguousarray(np.asarray(inputs["h"], dtype=np.float32))
    WQ = np.asarray(inputs["WQ"], dtype=np.float32) * WSC
    WK = np.asarray(inputs["WK"], dtype=np.float32) * WSC
    WV = np.asarray(inputs["WV"], dtype=np.float32) * WSC
    fcw = np.asarray(inputs["fc_w"], dtype=np.float32)
    fcb = np.asarray(inputs["fc_b"], dtype=np.float32)

    WQT = np.ascontiguousarray(WQ.T).astype(f8np)
    WKT = np.ascontiguousarray(WK.T).astype(f8np)
    WVT = np.ascontiguousarray(WV.T).astype(f8np)
    FCWT = np.ascontiguousarray(fcw.T).astype(bf)
    FCBB = np.ascontiguousarray(
        np.broadcast_to(fcb.reshape(1, HID), (128, HID))).astype(np.float32)
    IDT = np.eye(128, dtype=np.float32).astype(bf)

    def paired_T(x):
        """[ROWS, 256] -> [128, 2*ROWS] fp8: xT e-chunks side by side."""
        xT = x.T.reshape(2, 128, ROWS).transpose(1, 0, 2)
        return np.ascontiguousarray(xT.reshape(128, 2 * ROWS)).astype(f8np)

    in_maps = []
    for c in range(NCORES):
        sl = slice(c * ROWS, (c + 1) * ROWS)
        qc = q[sl]
        m = {
            "qTP": paired_T(qc),
            "q": qc.astype(bf),
            "hTP": paired_T(h[sl]),
            "WQT": WQT, "WKT": WKT, "WVT": WVT,
            "FCWT": FCWT, "FCBB": FCBB, "IDT": IDT,
        }
        if apply0:
            m["N0W"] = np.ascontiguousarray(
                np.broadcast_to(np.asarray(inputs["norm0_w"], np.float32),
                                (128, HID)))
            m["N0B"] = np.ascontiguousarray(
                np.broadcast_to(np.asarray(inputs["norm0_b"], np.float32),
                                (128, HID)))
        in_maps.append(m)
    return in_maps


def _run(inputs, trace=False, tmpdir=None):
    from concourse import bass_utils

    n0w = np.asarray(inputs["norm0_w"], np.float32)
    n0b = np.asarray(inputs["norm0_b"], np.float32)
    n1w = np.asarray(inputs["norm1_w"], np.float32)
    n1b = np.asarray(inputs["norm1_b"], np.float32)
    apply0 = not (np.allclose(n0w, 1.0) and np.allclose(n0b, 0.0))
    apply1 = not (np.allclose(n1w, 1.0) and np.allclose(n1b, 0.0))

    nc = _get_nc(apply0)
    in_maps = _shard(inputs, apply0)
    res = bass_utils.run_bass_kernel_spmd(
        nc, in_maps, core_ids=list(range(NCORES)), trace=trace,
        tmpdir=tmpdir)
    out = np.concatenate([np.asarray(res.results[c]["out"])
                          for c in range(NCORES)], axis=0)
    if apply1:
        out = out * n1w[None, :] + n1b[None, :]
    return out.astype(np.float32), res


def kernel(**inputs):
    out, _ = _run(inputs, trace=False)
    return out
